# revision 17
# baseline (speedup 1.0000x reference)
"""Trainium2 Bass kernel for the CILRS-style command-conditioned driving head.

Strategy (pure data parallel across 8 NeuronCores + host-side MoE routing):
  - Rows are grouped by `command` on the host and dealt round-robin across the
    8 cores, so each core gets ~B/8 rows with 4 contiguous command groups.
    Group boundaries are padded up to a 256-column grid so every on-device
    matmul segment is >=256 columns (fp32r full-rate requirement).
  - Everything on device is computed feature-major ([feature, batch]): the
    batch lives on the PE moving (free) axis, weights are the stationary
    operand in their natural [in, out] layout, so no transposes are needed
    anywhere on device (feat is transposed once on the host).
  - All matmuls run in float32r (TF32-like, ~1e-4 relative rounding) at the
    full 1 element/cycle PE rate. PSUM accumulates in fp32. Bias + ReLU
    epilogues are split across the Activation and Vector engines and write
    float32r so downstream matmuls can legally consume them.
  - The command dispatch is resolved by host routing: each ctrl-head matmul
    covers exactly the column range of its command group, so only 1/4 of the
    dense ctrl-head FLOPs are spent (plus small padding).
  - The per-chunk stages are software-pipelined: the ctrl-head stage of chunk
    c-1 is interleaved into chunk c's ms/sp/join matmul stream, so the PE
    never stalls on an epilogue and the HAM clock gate stays at full rate.

kernel(**inputs) takes the FULL unsharded inputs (as produced by
reference.setup_inputs()) and returns (v_p [B,1], throttle [B], brake [B],
steering [B]) as float32, matching the reference output tuple.
"""

import functools
import os
from contextlib import ExitStack

import numpy as np

import concourse.bacc as bacc
import concourse.tile as tile
from concourse import mybir
from concourse.bass_utils import run_bass_kernel_spmd

N_CORES = 8
P = 128
CH = 512  # batch-column chunk width (one PSUM bank of fp32)
FEAT, SPD_H, CTRL_H, N_CMD = 512, 128, 256, 4
KF = FEAT // P  # feat k-tiles (4)
F32 = mybir.dt.float32
F32R = mybir.dt.float32r

AF = mybir.ActivationFunctionType
ALU = mybir.AluOpType

# Results of the most recent kernel() call (for test harness introspection).
LAST_RESULTS = None


def _round_fp32r(x: np.ndarray) -> np.ndarray:
    """Round-to-nearest fp32 -> fp32r (11-bit mantissa, low 12 bits zero)."""
    u = np.ascontiguousarray(x, dtype=np.float32).view(np.uint32).astype(np.uint64)
    r = (u + 0x800) & 0xFFFF_F000
    return r.astype(np.uint32).view(np.float32)


# ---------------------------------------------------------------------------
# Weight packs: three [128, *] fp32 arrays (split so the first matmuls don't
# wait on the big ctrl-head weights). Addresses are (pack, col).
# ---------------------------------------------------------------------------
class _WPack:
    def __init__(self):
        self.cur = [0, 0, 0, 0]
        self.off = {}

    def alloc(self, pack, name, cols):
        self.off[name] = (pack, self.cur[pack])
        self.cur[pack] += cols
        return self.off[name]


_WP = _WPack()
# pack 0: needed by the very first matmuls + every bias
_MS1 = _WP.alloc(0, "ms1", P)           # [1,128] in row 0
_MS2 = _WP.alloc(0, "ms2", P)
_MS3 = _WP.alloc(0, "ms3", P)
_SP1 = _WP.alloc(1, "sp1", KF * CTRL_H)
_B_MS1 = _WP.alloc(0, "b_ms1", 1)
_B_MS2 = _WP.alloc(0, "b_ms2", 1)
_B_MS3 = _WP.alloc(0, "b_ms3", 1)
_B_JOIN = _WP.alloc(0, "b_join", KF)
_B_SP1 = _WP.alloc(0, "b_sp1", 2)
_B_SP2 = _WP.alloc(0, "b_sp2", 2)
_B_SP3 = _WP.alloc(0, "b_sp3", 1)
_B_C1 = _WP.alloc(0, "b_c1", N_CMD * 2)
_B_C2 = _WP.alloc(0, "b_c2", N_CMD * 2)
_B_C3 = _WP.alloc(0, "b_c3", N_CMD)      # rows 1..3 (row 0 = pad lane)
# pack 2: needed later in chunk 0
_JOIN = _WP.alloc(2, "join", 5 * FEAT)
_SP2 = _WP.alloc(2, "sp2", 2 * CTRL_H)
_SP3 = _WP.alloc(2, "sp3", 2 * 2)
# pack 3: ctrl heads, needed from chunk 1 on
_C1 = _WP.alloc(3, "c1", N_CMD * KF * CTRL_H)
_C2 = _WP.alloc(3, "c2", N_CMD * 2 * CTRL_H)
_C3 = _WP.alloc(3, "c3", N_CMD * 2 * 4)
_WC = tuple(_WP.cur)


def _build_wpacks(i):
    ws = [np.zeros((P, c), dtype=np.float32) for c in _WC]

    def put_lhst(dst, mat, m_cols):
        pack, base = dst
        w = ws[pack]
        K = mat.shape[0]
        for k in range((K + P - 1) // P):
            blk = mat[k * P:(k + 1) * P]
            w[: blk.shape[0], base + k * m_cols: base + k * m_cols + mat.shape[1]] = blk

    def col(dst, extra=0):
        pack, base = dst
        return ws[pack], base + extra

    put_lhst(_MS1, np.asarray(i["ms_w1"], np.float32), P)
    put_lhst(_MS2, np.asarray(i["ms_w2"], np.float32), P)
    put_lhst(_MS3, np.asarray(i["ms_w3"], np.float32), P)
    put_lhst(_JOIN, np.asarray(i["join_w"], np.float32), FEAT)
    put_lhst(_SP1, np.asarray(i["sp_w1"], np.float32), CTRL_H)
    put_lhst(_SP2, np.asarray(i["sp_w2"], np.float32), CTRL_H)
    put_lhst(_SP3, np.asarray(i["sp_w3"], np.float32), 2)
    cw1 = np.asarray(i["ctrl_w1"], np.float32)
    cw2 = np.asarray(i["ctrl_w2"], np.float32)
    cw3 = np.asarray(i["ctrl_w3"], np.float32)
    for e in range(N_CMD):
        put_lhst((_C1[0], _C1[1] + e * KF * CTRL_H), cw1[e], CTRL_H)
        put_lhst((_C2[0], _C2[1] + e * 2 * CTRL_H), cw2[e], CTRL_H)
        c3p = np.zeros((2 * P, 4), dtype=np.float32)
        c3p[:, 1:4] = cw3[e]
        put_lhst((_C3[0], _C3[1] + e * 8), c3p, 4)

    w, b = col(_B_MS1); w[:, b] = np.asarray(i["ms_b1"], np.float32)
    w, b = col(_B_MS2); w[:, b] = np.asarray(i["ms_b2"], np.float32)
    w, b = col(_B_MS3); w[:, b] = np.asarray(i["ms_b3"], np.float32)
    jb = np.asarray(i["join_b"], np.float32)
    sb1 = np.asarray(i["sp_b1"], np.float32)
    sb2 = np.asarray(i["sp_b2"], np.float32)
    for m in range(KF):
        w, b = col(_B_JOIN, m); w[:, b] = jb[m * P:(m + 1) * P]
    for m in range(2):
        w, b = col(_B_SP1, m); w[:, b] = sb1[m * P:(m + 1) * P]
        w, b = col(_B_SP2, m); w[:, b] = sb2[m * P:(m + 1) * P]
    w, b = col(_B_SP3); w[0, b] = np.asarray(i["sp_b3"], np.float32)[0]
    cb1 = np.asarray(i["ctrl_b1"], np.float32)
    cb2 = np.asarray(i["ctrl_b2"], np.float32)
    cb3 = np.asarray(i["ctrl_b3"], np.float32)
    for e in range(N_CMD):
        for m in range(2):
            w, b = col(_B_C1, e * 2 + m); w[:, b] = cb1[e, m * P:(m + 1) * P]
            w, b = col(_B_C2, e * 2 + m); w[:, b] = cb2[e, m * P:(m + 1) * P]
        w, b = col(_B_C3, e); w[1:4, b] = cb3[e]
    return [_round_fp32r(w) for w in ws]


# ---------------------------------------------------------------------------
# Bass kernel builder (cached per (T, head-boundary) signature).
# ---------------------------------------------------------------------------
@functools.lru_cache(maxsize=8)
def _build_nc(T: int, bounds: tuple):
    """bounds: (o1, o2, o3) column offsets where heads 1,2,3 start; head 0
    starts at 0, head 3 ends at T. All offsets are multiples of 256."""
    assert T % 2 == 0
    # chunk widths: full 512s, then split the remainder into 1-2 even chunks
    # of >=256 columns so every matmul keeps the fp32r full-rate window
    widths = []
    rem = T
    while rem >= CH + 256 or rem == CH:
        widths.append(CH)
        rem -= CH
    if rem > CH:
        a = (rem // 2 + 1) & ~1
        widths.append(a)
        widths.append(rem - a)
    elif rem > 0:
        widths.append(rem)
    starts = [0]
    for w in widths:
        starts.append(starts[-1] + w)
    n_chunks = len(widths)
    offs = (0,) + tuple(bounds) + (T,)

    nc = bacc.Bacc("TRN2", target_bir_lowering=False, debug=False,
                   num_devices=N_CORES)
    featd = nc.declare_dram_parameter("featT", [P, KF, T], F32R, isOutput=False)
    sprd = nc.declare_dram_parameter("sprow", [1, T], F32R, isOutput=False)
    wpd = [nc.declare_dram_parameter(f"wpack{k}", [P, _WC[k]], F32R,
                                     isOutput=False) for k in range(4)]
    outd = nc.declare_dram_parameter("out", [4, T], F32, isOutput=True)

    def segs_of(c):
        lo_c, hi_c = starts[c], starts[c + 1]
        out = []
        for e in range(N_CMD):
            lo, hi = max(lo_c, offs[e]), min(hi_c, offs[e + 1])
            if lo < hi:
                out.append((lo - lo_c, hi - lo_c, e))
        return out

    with tile.TileContext(nc) as tc, ExitStack() as ctx:
        wpool = ctx.enter_context(tc.tile_pool(name="w", bufs=1))
        fpool = ctx.enter_context(tc.tile_pool(name="f", bufs=3))
        apool = ctx.enter_context(tc.tile_pool(name="a", bufs=2))
        opool = ctx.enter_context(tc.tile_pool(name="o", bufs=3))
        pp = ctx.enter_context(tc.tile_pool(name="pp", bufs=8, space="PSUM"))

        spt = wpool.tile([1, T], F32R)
        nc.sync.dma_start(spt[:], sprd[:])
        wp = [wpool.tile([P, _WC[k]], F32R, name=f"wp{k}") for k in range(4)]
        # pack0 (small, needed by the first matmuls) loads up front; the big
        # packs are issued on the SWDGE ring inside chunk 0 so the first feat
        # chunks get the startup HBM bandwidth.
        nc.scalar.dma_start(wp[0][:], wpd[0][:])
        nc.scalar.dma_start(wp[1][:], wpd[1][:])

        def wsl(dst, extra, m):
            pack, base = dst
            return wp[pack][:, base + extra: base + extra + m]

        def bias(dst, extra=0):
            pack, base = dst
            return wp[pack][:, base + extra: base + extra + 1].bitcast(F32)

        def brow(dst, extra, r0, r1):
            pack, base = dst
            return wp[pack][r0:r1, base + extra: base + extra + 1].bitcast(F32)

        npsum = [0]

        def psum(p_, n_):
            npsum[0] += 1
            return pp.tile([p_, n_], F32, tag="ps", name=f"ps{npsum[0]}",
                           padded_shape=[P, CH])

        mm = nc.tensor.matmul
        act = nc.scalar.activation

        st = {}  # per-chunk carried state


        def front(c):
            """ms/sp/join matmuls of chunk c, with hooks for ctrl(c-1)."""
            cw = widths[c]
            sl = slice(starts[c], starts[c + 1])
            fa = fpool.tile([P, KF, CH], F32R, tag="feat", name=f"fa{c}")
            nc.sync.dma_start(fa[:, 0:2, :cw], featd[:, 0:2, sl])
            nc.sync.dma_start(fa[:, 2:4, :cw], featd[:, 2:4, sl])
            s = st[c] = {}

            # A: ms1 (outer product from speed row)
            p = psum(P, cw)
            mm(p[:], wp[0][0:1, _MS1[1]:_MS1[1] + P], spt[0:1, sl],
               start=True, stop=True)
            h1 = apool.tile([P, CH], F32R, tag="h1", name=f"h1_{c}")
            act(h1[:, :cw], p[:], AF.Relu, bias=bias(_B_MS1))

            # B: sp1
            s1 = []
            for m in range(2):
                p = psum(P, cw)
                for k in range(KF):
                    mm(p[:], wsl(_SP1, k * CTRL_H + m * P, P), fa[:, k, :cw],
                       start=(k == 0), stop=(k == KF - 1))
                t = apool.tile([P, CH], F32R, tag=f"s1{m}", name=f"s1{m}_{c}")
                act(t[:, :cw], p[:], AF.Relu, bias=bias(_B_SP1, m))
                s1.append(t)
            if c == 0:
                nc.gpsimd.dma_start(wp[2][:], wpd[2][:])

            # D: ms2
            p = psum(P, cw)
            mm(p[:], wsl(_MS2, 0, P), h1[:, :cw], start=True, stop=True)
            h2 = apool.tile([P, CH], F32R, tag="h2", name=f"h2_{c}")
            act(h2[:, :cw], p[:], AF.Relu, bias=bias(_B_MS2))

            # C: ctrl1 of previous chunk
            ctrl1(c - 1)

            # G: ms3
            p = psum(P, cw)
            mm(p[:], wsl(_MS3, 0, P), h2[:, :cw], start=True, stop=True)
            v = apool.tile([P, CH], F32R, tag="v", name=f"v_{c}")
            act(v[:, :cw], p[:], AF.Identity, bias=bias(_B_MS3))

            # H: sp2
            s2 = []
            for m in range(2):
                p = psum(P, cw)
                for k in range(2):
                    mm(p[:], wsl(_SP2, k * CTRL_H + m * P, P), s1[k][:, :cw],
                       start=(k == 0), stop=(k == 1))
                t = apool.tile([P, CH], F32R, tag=f"s2{m}", name=f"s2{m}_{c}")
                act(t[:, :cw], p[:], AF.Relu, bias=bias(_B_SP2, m))
                s2.append(t)
            if c == 0:
                nc.gpsimd.dma_start(wp[3][:], wpd[3][:])

            # F: ctrl2 of previous chunk
            ctrl2(c - 1)

            # E+I: join (k0..3 from feat, then k4 from v)
            jps = []
            for m in range(KF):
                p = psum(P, cw)
                for k in range(KF):
                    mm(p[:], wsl(_JOIN, k * FEAT + m * P, P), fa[:, k, :cw],
                       start=(k == 0), stop=False)
                jps.append(p)
            j = []
            for m in range(KF):
                mm(jps[m][:], wsl(_JOIN, KF * FEAT + m * P, P), v[:, :cw],
                   start=False, stop=True)
                t = apool.tile([P, CH], F32R, tag=f"j{m}", name=f"j{m}_{c}")
                nc.vector.tensor_scalar_add(t[:, :cw], jps[m][:], bias(_B_JOIN, m))
                j.append(t)
            s["j"] = j

            # J: ctrl3 of previous chunk (+ its output DMAs)
            ctrl3(c - 1)

            # K: sp3
            p3 = psum(2, cw)
            for k in range(2):
                mm(p3[:], wsl(_SP3, 2 * k, 2), s2[k][:, :cw],
                   start=(k == 0), stop=(k == 1))
            vp_t = apool.tile([1, CH], F32, tag="vp", name=f"vp_{c}")
            act(vp_t[0:1, :cw], p3[0:1, :], AF.Identity, bias=brow(_B_SP3, 0, 0, 1))
            s["vp"] = vp_t

        def ctrl1(c):
            if c < 0:
                return
            s = st[c]
            j = s["j"]
            s["t1"] = {}
            for (lo, hi, e) in segs_of(c):
                n = hi - lo
                t1 = []
                for m in range(2):
                    p = psum(P, n)
                    for k in range(KF):
                        mm(p[:], wsl(_C1, (e * KF + k) * CTRL_H + m * P, P),
                           j[k][:, lo:hi],
                           start=(k == 0), stop=(k == KF - 1))
                    t = apool.tile([P, CH], F32R, tag=f"t1{m}",
                                   name=f"t1{m}_{c}_{lo}")
                    nc.vector.tensor_scalar(t[:, :n], p[:],
                                            bias(_B_C1, e * 2 + m), 0.0,
                                            ALU.add, ALU.max)
                    t1.append(t)
                s["t1"][lo] = t1

        def ctrl2(c):
            if c < 0:
                return
            s = st[c]
            s["t2"] = {}
            for (lo, hi, e) in segs_of(c):
                n = hi - lo
                t1 = s["t1"][lo]
                t2 = []
                for m in range(2):
                    p = psum(P, n)
                    for k in range(2):
                        mm(p[:], wsl(_C2, (e * 2 + k) * CTRL_H + m * P, P),
                           t1[k][:, :n], start=(k == 0), stop=(k == 1))
                    t = apool.tile([P, CH], F32R, tag=f"t2{m}",
                                   name=f"t2{m}_{c}_{lo}")
                    nc.vector.tensor_scalar(t[:, :n], p[:],
                                            bias(_B_C2, e * 2 + m), 0.0,
                                            ALU.add, ALU.max)
                    t2.append(t)
                s["t2"][lo] = t2

        def ctrl3(c):
            if c < 0:
                return
            s = st[c]
            cw = widths[c]
            sl = slice(starts[c], starts[c + 1])
            outS = opool.tile([4, CH], F32, tag="outS", name=f"outS_{c}")
            outT = opool.tile([4, CH], F32, tag="outT", name=f"outT_{c}")
            for (lo, hi, e) in segs_of(c):
                n = hi - lo
                t2 = s["t2"][lo]
                pc = psum(4, n)
                for k in range(2):
                    mm(pc[:], wsl(_C3, e * 8 + k * 4, 4), t2[k][:, :n],
                       start=(k == 0), stop=(k == 1))
                # psum rows: 0=pad, 1=act0(throttle), 2=act1(steer), 3=act2(brake)
                act(outS[0:4, lo:hi], pc[0:4, :], AF.Sigmoid,
                    bias=brow(_B_C3, e, 0, 4))
                act(outT[0:4, lo:hi], pc[0:4, :], AF.Tanh,
                    bias=brow(_B_C3, e, 0, 4))
            # out rows: 0=v_p, 1=throttle, 2=steering, 3=brake
            nc.sync.dma_start(outd[0:1, sl], st[c]["vp"][0:1, :cw])
            nc.sync.dma_start(outd[1:2, sl], outS[1:2, :cw])
            nc.sync.dma_start(outd[2:3, sl], outT[2:3, :cw])
            nc.sync.dma_start(outd[3:4, sl], outS[3:4, :cw])
            del st[c]["t1"], st[c]["t2"], st[c]["j"]

        for c in range(n_chunks):
            front(c)
        ctrl1(n_chunks - 1)
        ctrl2(n_chunks - 1)
        ctrl3(n_chunks - 1)

    nc.compile()
    return nc


def _roundup(x, m):
    return (x + m - 1) // m * m


def kernel(**inputs) -> tuple:
    global LAST_RESULTS
    feat = np.asarray(inputs["feat"], np.float32)
    speed = np.asarray(inputs["speed"], np.float32)
    command = np.asarray(inputs["command"]).astype(np.int64)
    B = feat.shape[0]

    # ---- host routing: group rows by command, deal round-robin over cores --
    per_core_groups = [[None] * N_CMD for _ in range(N_CORES)]
    for e in range(N_CMD):
        idx = np.nonzero(command == e)[0]
        for cid in range(N_CORES):
            per_core_groups[cid][e] = idx[cid::N_CORES]

    counts = np.array([[len(per_core_groups[cid][e]) for e in range(N_CMD)]
                       for cid in range(N_CORES)])
    caps = counts.max(axis=0)  # per-head capacity across cores
    # Head boundaries: even columns; avoid splitting a chunk into two
    # mid-sized pieces (both would lose the fp32r full-rate N>=256 window) by
    # pushing such boundaries up to the next 256 multiple.
    offs = [0]
    for e in range(N_CMD):
        b = _roundup(offs[e] + int(caps[e]), 2)
        if 130 < (b % CH) < 382:
            b = _roundup(b, 256)
        offs.append(b)
    T = max(_roundup(offs[N_CMD], 2), 512)
    bounds = tuple(offs[1:4])

    wpacks = _build_wpacks(inputs)

    in_maps = []
    for cid in range(N_CORES):
        rows = np.zeros((T, FEAT), dtype=np.float32)
        spr = np.zeros(T, dtype=np.float32)
        for e in range(N_CMD):
            g = per_core_groups[cid][e]
            rows[offs[e]: offs[e] + len(g)] = feat[g]
            spr[offs[e]: offs[e] + len(g)] = speed[g]
        featT = _round_fp32r(rows.T)                      # [512, T]
        featd = np.ascontiguousarray(
            featT.reshape(KF, P, T).transpose(1, 0, 2))   # [128, 4, T]
        in_maps.append({
            "featT": featd,
            "sprow": _round_fp32r(spr)[None, :],
            "wpack0": wpacks[0],
            "wpack1": wpacks[1],
            "wpack2": wpacks[2],
            "wpack3": wpacks[3],
        })

    nc = _build_nc(T, bounds)
    trace = os.environ.get("KERNEL_TRACE", "") == "1"
    res = run_bass_kernel_spmd(nc, in_maps, core_ids=list(range(N_CORES)),
                               trace=trace)
    LAST_RESULTS = res

    v_p = np.zeros((B, 1), dtype=np.float32)
    throttle = np.zeros(B, dtype=np.float32)
    brake = np.zeros(B, dtype=np.float32)
    steering = np.zeros(B, dtype=np.float32)
    for cid in range(N_CORES):
        o = res.results[cid]["out"]  # [4, T]
        for e in range(N_CMD):
            g = per_core_groups[cid][e]
            sl = slice(offs[e], offs[e] + len(g))
            v_p[g, 0] = o[0, sl]
            throttle[g] = o[1, sl]
            steering[g] = o[2, sl]
            brake[g] = o[3, sl]
    return v_p, throttle, brake, steering


# revision 18
# speedup vs baseline: 1.0386x; 1.0386x over previous
"""Trainium2 Bass kernel for the CILRS-style command-conditioned driving head.

Strategy (pure data parallel across 8 NeuronCores + host-side MoE routing):
  - Rows are grouped by `command` on the host and dealt round-robin across the
    8 cores, so each core gets ~B/8 rows with 4 contiguous command groups.
    Group boundaries are padded up to a 256-column grid so every on-device
    matmul segment is >=256 columns (fp32r full-rate requirement).
  - Everything on device is computed feature-major ([feature, batch]): the
    batch lives on the PE moving (free) axis, weights are the stationary
    operand in their natural [in, out] layout, so no transposes are needed
    anywhere on device (feat is transposed once on the host).
  - All matmuls run in float32r (TF32-like, ~1e-4 relative rounding) at the
    full 1 element/cycle PE rate. PSUM accumulates in fp32. Bias + ReLU
    epilogues are split across the Activation and Vector engines and write
    float32r so downstream matmuls can legally consume them.
  - The command dispatch is resolved by host routing: each ctrl-head matmul
    covers exactly the column range of its command group, so only 1/4 of the
    dense ctrl-head FLOPs are spent (plus small padding).
  - The per-chunk stages are software-pipelined: the ctrl-head stage of chunk
    c-1 is interleaved into chunk c's ms/sp/join matmul stream, so the PE
    never stalls on an epilogue and the HAM clock gate stays at full rate.

kernel(**inputs) takes the FULL unsharded inputs (as produced by
reference.setup_inputs()) and returns (v_p [B,1], throttle [B], brake [B],
steering [B]) as float32, matching the reference output tuple.
"""

import functools
import os
from contextlib import ExitStack

import numpy as np

import concourse.bacc as bacc
import concourse.tile as tile
from concourse import mybir
from concourse.bass_utils import run_bass_kernel_spmd

N_CORES = 8
P = 128
CH = 512  # batch-column chunk width (one PSUM bank of fp32)
FEAT, SPD_H, CTRL_H, N_CMD = 512, 128, 256, 4
KF = FEAT // P  # feat k-tiles (4)
F32 = mybir.dt.float32
F32R = mybir.dt.float32r

AF = mybir.ActivationFunctionType
ALU = mybir.AluOpType

# Results of the most recent kernel() call (for test harness introspection).
LAST_RESULTS = None


def _round_fp32r(x: np.ndarray) -> np.ndarray:
    """Round-to-nearest fp32 -> fp32r (11-bit mantissa, low 12 bits zero)."""
    u = np.ascontiguousarray(x, dtype=np.float32).view(np.uint32).astype(np.uint64)
    r = (u + 0x800) & 0xFFFF_F000
    return r.astype(np.uint32).view(np.float32)


# ---------------------------------------------------------------------------
# Weight packs: three [128, *] fp32 arrays (split so the first matmuls don't
# wait on the big ctrl-head weights). Addresses are (pack, col).
# ---------------------------------------------------------------------------
class _WPack:
    def __init__(self):
        self.cur = [0, 0, 0, 0]
        self.off = {}

    def alloc(self, pack, name, cols):
        self.off[name] = (pack, self.cur[pack])
        self.cur[pack] += cols
        return self.off[name]


_WP = _WPack()
# pack 0: needed by the very first matmuls + every bias
_MS1 = _WP.alloc(0, "ms1", P)           # [1,128] in row 0
_MS2 = _WP.alloc(0, "ms2", P)
_MS3 = _WP.alloc(0, "ms3", P)
_SP1 = _WP.alloc(1, "sp1", KF * CTRL_H)
_B_MS1 = _WP.alloc(0, "b_ms1", 1)
_B_MS2 = _WP.alloc(0, "b_ms2", 1)
_B_MS3 = _WP.alloc(0, "b_ms3", 1)
_B_JOIN = _WP.alloc(0, "b_join", KF)
_B_SP1 = _WP.alloc(0, "b_sp1", 2)
_B_SP2 = _WP.alloc(0, "b_sp2", 2)
_B_SP3 = _WP.alloc(0, "b_sp3", 1)
_B_C1 = _WP.alloc(0, "b_c1", N_CMD * 2)
_B_C2 = _WP.alloc(0, "b_c2", N_CMD * 2)
_B_C3 = _WP.alloc(0, "b_c3", N_CMD)      # rows 1..3 (row 0 = pad lane)
# pack 2: needed later in chunk 0
_JOIN = _WP.alloc(2, "join", 5 * FEAT)
_SP2 = _WP.alloc(2, "sp2", 2 * CTRL_H)
_SP3 = _WP.alloc(2, "sp3", 2 * 2)
# pack 3: ctrl heads, needed from chunk 1 on
_C1 = _WP.alloc(3, "c1", N_CMD * KF * CTRL_H)
_C2 = _WP.alloc(3, "c2", N_CMD * 2 * CTRL_H)
_C3 = _WP.alloc(3, "c3", N_CMD * 2 * 4)
_WC = tuple(_WP.cur)


def _build_wpacks(i):
    ws = [np.zeros((P, c), dtype=np.float32) for c in _WC]

    def put_lhst(dst, mat, m_cols):
        pack, base = dst
        w = ws[pack]
        K = mat.shape[0]
        for k in range((K + P - 1) // P):
            blk = mat[k * P:(k + 1) * P]
            w[: blk.shape[0], base + k * m_cols: base + k * m_cols + mat.shape[1]] = blk

    def col(dst, extra=0):
        pack, base = dst
        return ws[pack], base + extra

    put_lhst(_MS1, np.asarray(i["ms_w1"], np.float32), P)
    put_lhst(_MS2, np.asarray(i["ms_w2"], np.float32), P)
    put_lhst(_MS3, np.asarray(i["ms_w3"], np.float32), P)
    put_lhst(_JOIN, np.asarray(i["join_w"], np.float32), FEAT)
    put_lhst(_SP1, np.asarray(i["sp_w1"], np.float32), CTRL_H)
    put_lhst(_SP2, np.asarray(i["sp_w2"], np.float32), CTRL_H)
    put_lhst(_SP3, np.asarray(i["sp_w3"], np.float32), 2)
    cw1 = np.asarray(i["ctrl_w1"], np.float32)
    cw2 = np.asarray(i["ctrl_w2"], np.float32)
    cw3 = np.asarray(i["ctrl_w3"], np.float32)
    for e in range(N_CMD):
        put_lhst((_C1[0], _C1[1] + e * KF * CTRL_H), cw1[e], CTRL_H)
        put_lhst((_C2[0], _C2[1] + e * 2 * CTRL_H), cw2[e], CTRL_H)
        c3p = np.zeros((2 * P, 4), dtype=np.float32)
        c3p[:, 1:4] = cw3[e]
        put_lhst((_C3[0], _C3[1] + e * 8), c3p, 4)

    w, b = col(_B_MS1); w[:, b] = np.asarray(i["ms_b1"], np.float32)
    w, b = col(_B_MS2); w[:, b] = np.asarray(i["ms_b2"], np.float32)
    w, b = col(_B_MS3); w[:, b] = np.asarray(i["ms_b3"], np.float32)
    jb = np.asarray(i["join_b"], np.float32)
    sb1 = np.asarray(i["sp_b1"], np.float32)
    sb2 = np.asarray(i["sp_b2"], np.float32)
    for m in range(KF):
        w, b = col(_B_JOIN, m); w[:, b] = jb[m * P:(m + 1) * P]
    for m in range(2):
        w, b = col(_B_SP1, m); w[:, b] = sb1[m * P:(m + 1) * P]
        w, b = col(_B_SP2, m); w[:, b] = sb2[m * P:(m + 1) * P]
    w, b = col(_B_SP3); w[0, b] = np.asarray(i["sp_b3"], np.float32)[0]
    cb1 = np.asarray(i["ctrl_b1"], np.float32)
    cb2 = np.asarray(i["ctrl_b2"], np.float32)
    cb3 = np.asarray(i["ctrl_b3"], np.float32)
    for e in range(N_CMD):
        for m in range(2):
            w, b = col(_B_C1, e * 2 + m); w[:, b] = cb1[e, m * P:(m + 1) * P]
            w, b = col(_B_C2, e * 2 + m); w[:, b] = cb2[e, m * P:(m + 1) * P]
        w, b = col(_B_C3, e); w[1:4, b] = cb3[e]
    return [_round_fp32r(w) for w in ws]


# ---------------------------------------------------------------------------
# Bass kernel builder (cached per (T, head-boundary) signature).
# ---------------------------------------------------------------------------
@functools.lru_cache(maxsize=8)
def _build_nc(T: int, bounds: tuple):
    """bounds: (o1, o2, o3) column offsets where heads 1,2,3 start; head 0
    starts at 0, head 3 ends at T. All offsets are multiples of 256."""
    assert T % 2 == 0
    # chunk widths: full 512s, then split the remainder into 1-2 even chunks
    # of >=256 columns so every matmul keeps the fp32r full-rate window
    widths = []
    rem = T
    while rem >= CH + 256 or rem == CH:
        widths.append(CH)
        rem -= CH
    if rem > CH:
        a = (rem // 2 + 1) & ~1
        widths.append(a)
        widths.append(rem - a)
    elif rem > 0:
        widths.append(rem)
    starts = [0]
    for w in widths:
        starts.append(starts[-1] + w)
    n_chunks = len(widths)
    offs = (0,) + tuple(bounds) + (T,)

    nc = bacc.Bacc("TRN2", target_bir_lowering=False, debug=False,
                   num_devices=N_CORES)
    featd = nc.declare_dram_parameter("featT", [P, KF, T], F32R, isOutput=False)
    sprd = nc.declare_dram_parameter("sprow", [1, T], F32R, isOutput=False)
    wpd = [nc.declare_dram_parameter(f"wpack{k}", [P, _WC[k]], F32R,
                                     isOutput=False) for k in range(4)]
    outd = nc.declare_dram_parameter("out", [4, T], F32, isOutput=True)

    def segs_of(c):
        lo_c, hi_c = starts[c], starts[c + 1]
        out = []
        for e in range(N_CMD):
            lo, hi = max(lo_c, offs[e]), min(hi_c, offs[e + 1])
            if lo < hi:
                out.append((lo - lo_c, hi - lo_c, e))
        return out

    with tile.TileContext(nc) as tc, ExitStack() as ctx:
        wpool = ctx.enter_context(tc.tile_pool(name="w", bufs=1))
        fpool = ctx.enter_context(tc.tile_pool(name="f", bufs=3))
        apool = ctx.enter_context(tc.tile_pool(name="a", bufs=2))
        opool = ctx.enter_context(tc.tile_pool(name="o", bufs=3))
        pp = ctx.enter_context(tc.tile_pool(name="pp", bufs=8, space="PSUM"))

        spt = wpool.tile([1, T], F32R)
        nc.sync.dma_start(spt[:], sprd[:])
        wp = [wpool.tile([P, _WC[k]], F32R, name=f"wp{k}") for k in range(4)]
        # pack0 (small, needed by the first matmuls) loads up front; the big
        # packs are issued on the SWDGE ring inside chunk 0 so the first feat
        # chunks get the startup HBM bandwidth.
        nc.scalar.dma_start(wp[0][:], wpd[0][:])
        nc.scalar.dma_start(wp[1][:], wpd[1][:])

        def wsl(dst, extra, m):
            pack, base = dst
            return wp[pack][:, base + extra: base + extra + m]

        def bias(dst, extra=0):
            pack, base = dst
            return wp[pack][:, base + extra: base + extra + 1].bitcast(F32)

        def brow(dst, extra, r0, r1):
            pack, base = dst
            return wp[pack][r0:r1, base + extra: base + extra + 1].bitcast(F32)

        npsum = [0]

        def psum(p_, n_):
            npsum[0] += 1
            return pp.tile([p_, n_], F32, tag="ps", name=f"ps{npsum[0]}",
                           padded_shape=[P, CH])

        mm = nc.tensor.matmul
        act = nc.scalar.activation

        st = {}  # per-chunk carried state


        def front(c):
            """ms/sp/join matmuls of chunk c, with hooks for ctrl(c-1)."""
            cw = widths[c]
            sl = slice(starts[c], starts[c + 1])
            fa = fpool.tile([P, KF, CH], F32R, tag="feat", name=f"fa{c}")
            nc.sync.dma_start(fa[:, 0:2, :cw], featd[:, 0:2, sl])
            nc.sync.dma_start(fa[:, 2:4, :cw], featd[:, 2:4, sl])
            s = st[c] = {}

            # A: ms1 (outer product from speed row)
            p = psum(P, cw)
            mm(p[:], wp[0][0:1, _MS1[1]:_MS1[1] + P], spt[0:1, sl],
               start=True, stop=True)
            h1 = apool.tile([P, CH], F32R, tag="h1", name=f"h1_{c}")
            act(h1[:, :cw], p[:], AF.Relu, bias=bias(_B_MS1))

            # B: sp1
            s1 = []
            for m in range(2):
                p = psum(P, cw)
                for k in range(KF):
                    mm(p[:], wsl(_SP1, k * CTRL_H + m * P, P), fa[:, k, :cw],
                       start=(k == 0), stop=(k == KF - 1))
                t = apool.tile([P, CH], F32R, tag=f"s1{m}", name=f"s1{m}_{c}")
                act(t[:, :cw], p[:], AF.Relu, bias=bias(_B_SP1, m))
                s1.append(t)
            if c == 0:
                # gate the big packs behind the startup-critical DMAs so
                # chunk 0's feat gets the HBM bandwidth first (WAW touch)
                nc.vector.tensor_copy(wp[2][0:1, 0:2], wp[1][0:1, 0:2])
                nc.gpsimd.dma_start(wp[2][:], wpd[2][:])

            # D: ms2
            p = psum(P, cw)
            mm(p[:], wsl(_MS2, 0, P), h1[:, :cw], start=True, stop=True)
            h2 = apool.tile([P, CH], F32R, tag="h2", name=f"h2_{c}")
            act(h2[:, :cw], p[:], AF.Relu, bias=bias(_B_MS2))

            # C: ctrl1 of previous chunk
            ctrl1(c - 1)

            # G: ms3
            p = psum(P, cw)
            mm(p[:], wsl(_MS3, 0, P), h2[:, :cw], start=True, stop=True)
            v = apool.tile([P, CH], F32R, tag="v", name=f"v_{c}")
            act(v[:, :cw], p[:], AF.Identity, bias=bias(_B_MS3))

            # H: sp2
            s2 = []
            for m in range(2):
                p = psum(P, cw)
                for k in range(2):
                    mm(p[:], wsl(_SP2, k * CTRL_H + m * P, P), s1[k][:, :cw],
                       start=(k == 0), stop=(k == 1))
                t = apool.tile([P, CH], F32R, tag=f"s2{m}", name=f"s2{m}_{c}")
                act(t[:, :cw], p[:], AF.Relu, bias=bias(_B_SP2, m))
                s2.append(t)
            if c == 0:
                nc.vector.tensor_copy(wp[3][0:1, 0:2], fa[0:1, 0, 0:2])
                nc.gpsimd.dma_start(wp[3][:], wpd[3][:])

            # F: ctrl2 of previous chunk
            ctrl2(c - 1)

            # E+I: join (k0..3 from feat, then k4 from v)
            jps = []
            for m in range(KF):
                p = psum(P, cw)
                for k in range(KF):
                    mm(p[:], wsl(_JOIN, k * FEAT + m * P, P), fa[:, k, :cw],
                       start=(k == 0), stop=False)
                jps.append(p)
            j = []
            for m in range(KF):
                mm(jps[m][:], wsl(_JOIN, KF * FEAT + m * P, P), v[:, :cw],
                   start=False, stop=True)
                t = apool.tile([P, CH], F32R, tag=f"j{m}", name=f"j{m}_{c}")
                nc.vector.tensor_scalar_add(t[:, :cw], jps[m][:], bias(_B_JOIN, m))
                j.append(t)
            s["j"] = j

            # J: ctrl3 of previous chunk (+ its output DMAs)
            ctrl3(c - 1)

            # K: sp3
            p3 = psum(2, cw)
            for k in range(2):
                mm(p3[:], wsl(_SP3, 2 * k, 2), s2[k][:, :cw],
                   start=(k == 0), stop=(k == 1))
            vp_t = apool.tile([1, CH], F32, tag="vp", name=f"vp_{c}")
            act(vp_t[0:1, :cw], p3[0:1, :], AF.Identity, bias=brow(_B_SP3, 0, 0, 1))
            s["vp"] = vp_t

        def ctrl1(c):
            if c < 0:
                return
            s = st[c]
            j = s["j"]
            s["t1"] = {}
            for (lo, hi, e) in segs_of(c):
                n = hi - lo
                t1 = []
                for m in range(2):
                    p = psum(P, n)
                    for k in range(KF):
                        mm(p[:], wsl(_C1, (e * KF + k) * CTRL_H + m * P, P),
                           j[k][:, lo:hi],
                           start=(k == 0), stop=(k == KF - 1))
                    t = apool.tile([P, CH], F32R, tag=f"t1{m}",
                                   name=f"t1{m}_{c}_{lo}")
                    nc.vector.tensor_scalar(t[:, :n], p[:],
                                            bias(_B_C1, e * 2 + m), 0.0,
                                            ALU.add, ALU.max)
                    t1.append(t)
                s["t1"][lo] = t1

        def ctrl2(c):
            if c < 0:
                return
            s = st[c]
            s["t2"] = {}
            for (lo, hi, e) in segs_of(c):
                n = hi - lo
                t1 = s["t1"][lo]
                t2 = []
                for m in range(2):
                    p = psum(P, n)
                    for k in range(2):
                        mm(p[:], wsl(_C2, (e * 2 + k) * CTRL_H + m * P, P),
                           t1[k][:, :n], start=(k == 0), stop=(k == 1))
                    t = apool.tile([P, CH], F32R, tag=f"t2{m}",
                                   name=f"t2{m}_{c}_{lo}")
                    nc.vector.tensor_scalar(t[:, :n], p[:],
                                            bias(_B_C2, e * 2 + m), 0.0,
                                            ALU.add, ALU.max)
                    t2.append(t)
                s["t2"][lo] = t2

        def ctrl3(c):
            if c < 0:
                return
            s = st[c]
            cw = widths[c]
            sl = slice(starts[c], starts[c + 1])
            outS = opool.tile([4, CH], F32, tag="outS", name=f"outS_{c}")
            outT = opool.tile([4, CH], F32, tag="outT", name=f"outT_{c}")
            for (lo, hi, e) in segs_of(c):
                n = hi - lo
                t2 = s["t2"][lo]
                pc = psum(4, n)
                for k in range(2):
                    mm(pc[:], wsl(_C3, e * 8 + k * 4, 4), t2[k][:, :n],
                       start=(k == 0), stop=(k == 1))
                # psum rows: 0=pad, 1=act0(throttle), 2=act1(steer), 3=act2(brake)
                act(outS[0:4, lo:hi], pc[0:4, :], AF.Sigmoid,
                    bias=brow(_B_C3, e, 0, 4))
                act(outT[0:4, lo:hi], pc[0:4, :], AF.Tanh,
                    bias=brow(_B_C3, e, 0, 4))
            # out rows: 0=v_p, 1=throttle, 2=steering, 3=brake
            nc.sync.dma_start(outd[0:1, sl], st[c]["vp"][0:1, :cw])
            nc.sync.dma_start(outd[1:2, sl], outS[1:2, :cw])
            nc.sync.dma_start(outd[2:3, sl], outT[2:3, :cw])
            nc.sync.dma_start(outd[3:4, sl], outS[3:4, :cw])
            del st[c]["t1"], st[c]["t2"], st[c]["j"]

        for c in range(n_chunks):
            front(c)
        ctrl1(n_chunks - 1)
        ctrl2(n_chunks - 1)
        ctrl3(n_chunks - 1)

    nc.compile()
    return nc


def _roundup(x, m):
    return (x + m - 1) // m * m


def kernel(**inputs) -> tuple:
    global LAST_RESULTS
    feat = np.asarray(inputs["feat"], np.float32)
    speed = np.asarray(inputs["speed"], np.float32)
    command = np.asarray(inputs["command"]).astype(np.int64)
    B = feat.shape[0]

    # ---- host routing: group rows by command, deal round-robin over cores --
    per_core_groups = [[None] * N_CMD for _ in range(N_CORES)]
    for e in range(N_CMD):
        idx = np.nonzero(command == e)[0]
        for cid in range(N_CORES):
            per_core_groups[cid][e] = idx[cid::N_CORES]

    counts = np.array([[len(per_core_groups[cid][e]) for e in range(N_CMD)]
                       for cid in range(N_CORES)])
    caps = counts.max(axis=0)  # per-head capacity across cores
    # Head boundaries: even columns; avoid splitting a chunk into two
    # mid-sized pieces (both would lose the fp32r full-rate N>=256 window) by
    # pushing such boundaries up to the next 256 multiple.
    offs = [0]
    for e in range(N_CMD):
        b = _roundup(offs[e] + int(caps[e]), 2)
        if 130 < (b % CH) < 382:
            b = _roundup(b, 256)
        offs.append(b)
    T = max(_roundup(offs[N_CMD], 2), 512)
    bounds = tuple(offs[1:4])

    wpacks = _build_wpacks(inputs)

    in_maps = []
    for cid in range(N_CORES):
        rows = np.zeros((T, FEAT), dtype=np.float32)
        spr = np.zeros(T, dtype=np.float32)
        for e in range(N_CMD):
            g = per_core_groups[cid][e]
            rows[offs[e]: offs[e] + len(g)] = feat[g]
            spr[offs[e]: offs[e] + len(g)] = speed[g]
        featT = _round_fp32r(rows.T)                      # [512, T]
        featd = np.ascontiguousarray(
            featT.reshape(KF, P, T).transpose(1, 0, 2))   # [128, 4, T]
        in_maps.append({
            "featT": featd,
            "sprow": _round_fp32r(spr)[None, :],
            "wpack0": wpacks[0],
            "wpack1": wpacks[1],
            "wpack2": wpacks[2],
            "wpack3": wpacks[3],
        })

    nc = _build_nc(T, bounds)
    trace = os.environ.get("KERNEL_TRACE", "") == "1"
    res = run_bass_kernel_spmd(nc, in_maps, core_ids=list(range(N_CORES)),
                               trace=trace)
    LAST_RESULTS = res

    v_p = np.zeros((B, 1), dtype=np.float32)
    throttle = np.zeros(B, dtype=np.float32)
    brake = np.zeros(B, dtype=np.float32)
    steering = np.zeros(B, dtype=np.float32)
    for cid in range(N_CORES):
        o = res.results[cid]["out"]  # [4, T]
        for e in range(N_CMD):
            g = per_core_groups[cid][e]
            sl = slice(offs[e], offs[e] + len(g))
            v_p[g, 0] = o[0, sl]
            throttle[g] = o[1, sl]
            steering[g] = o[2, sl]
            brake[g] = o[3, sl]
    return v_p, throttle, brake, steering


# revision 19
# speedup vs baseline: 1.0566x; 1.0173x over previous
"""Trainium2 Bass kernel for the CILRS-style command-conditioned driving head.

Strategy (pure data parallel across 8 NeuronCores + host-side MoE routing):
  - Rows are grouped by `command` on the host and dealt round-robin across the
    8 cores, so each core gets ~B/8 rows with 4 contiguous command groups.
    Group boundaries are padded up to a 256-column grid so every on-device
    matmul segment is >=256 columns (fp32r full-rate requirement).
  - Everything on device is computed feature-major ([feature, batch]): the
    batch lives on the PE moving (free) axis, weights are the stationary
    operand in their natural [in, out] layout, so no transposes are needed
    anywhere on device (feat is transposed once on the host).
  - All matmuls run in float32r (TF32-like, ~1e-4 relative rounding) at the
    full 1 element/cycle PE rate. PSUM accumulates in fp32. Bias + ReLU
    epilogues are split across the Activation and Vector engines and write
    float32r so downstream matmuls can legally consume them.
  - The command dispatch is resolved by host routing: each ctrl-head matmul
    covers exactly the column range of its command group, so only 1/4 of the
    dense ctrl-head FLOPs are spent (plus small padding).
  - The per-chunk stages are software-pipelined: the ctrl-head stage of chunk
    c-1 is interleaved into chunk c's ms/sp/join matmul stream, so the PE
    never stalls on an epilogue and the HAM clock gate stays at full rate.

kernel(**inputs) takes the FULL unsharded inputs (as produced by
reference.setup_inputs()) and returns (v_p [B,1], throttle [B], brake [B],
steering [B]) as float32, matching the reference output tuple.
"""

import functools
import os
from contextlib import ExitStack

import numpy as np

import concourse.bacc as bacc
import concourse.tile as tile
from concourse import mybir
from concourse.bass_utils import run_bass_kernel_spmd

N_CORES = 8
P = 128
CH = 512  # batch-column chunk width (one PSUM bank of fp32)
FEAT, SPD_H, CTRL_H, N_CMD = 512, 128, 256, 4
KF = FEAT // P  # feat k-tiles (4)
F32 = mybir.dt.float32
F32R = mybir.dt.float32r

AF = mybir.ActivationFunctionType
ALU = mybir.AluOpType

# Results of the most recent kernel() call (for test harness introspection).
LAST_RESULTS = None


def _round_fp32r(x: np.ndarray) -> np.ndarray:
    """Round-to-nearest fp32 -> fp32r (11-bit mantissa, low 12 bits zero)."""
    u = np.ascontiguousarray(x, dtype=np.float32).view(np.uint32).astype(np.uint64)
    r = (u + 0x800) & 0xFFFF_F000
    return r.astype(np.uint32).view(np.float32)


# ---------------------------------------------------------------------------
# Weight packs: three [128, *] fp32 arrays (split so the first matmuls don't
# wait on the big ctrl-head weights). Addresses are (pack, col).
# ---------------------------------------------------------------------------
class _WPack:
    def __init__(self):
        self.cur = [0, 0, 0, 0, 0, 0, 0]
        self.off = {}

    def alloc(self, pack, name, cols):
        self.off[name] = (pack, self.cur[pack])
        self.cur[pack] += cols
        return self.off[name]


_WP = _WPack()
# pack 0: needed by the very first matmuls + every bias
_MS1 = _WP.alloc(0, "ms1", P)           # [1,128] in row 0
_MS2 = _WP.alloc(0, "ms2", P)
_MS3 = _WP.alloc(0, "ms3", P)
_SP1 = _WP.alloc(1, "sp1", KF * CTRL_H)
_B_MS1 = _WP.alloc(0, "b_ms1", 1)
_B_MS2 = _WP.alloc(0, "b_ms2", 1)
_B_MS3 = _WP.alloc(0, "b_ms3", 1)
_B_JOIN = _WP.alloc(0, "b_join", KF)
_B_SP1 = _WP.alloc(0, "b_sp1", 2)
_B_SP2 = _WP.alloc(0, "b_sp2", 2)
_B_SP3 = _WP.alloc(0, "b_sp3", 1)
_B_C1 = _WP.alloc(0, "b_c1", N_CMD * 2)
_B_C2 = _WP.alloc(0, "b_c2", N_CMD * 2)
_B_C3 = _WP.alloc(0, "b_c3", N_CMD)      # rows 1..3 (row 0 = pad lane)
# pack 2: needed later in chunk 0
_JOIN = _WP.alloc(2, "join", 5 * FEAT)
_SP2 = _WP.alloc(2, "sp2", 2 * CTRL_H)
_SP3 = _WP.alloc(2, "sp3", 2 * 2)
# packs 3..6: ctrl head e (staggered loads; head e is first needed at the
# chunk containing column offs[e])
_C1, _C2, _C3 = [], [], []
for _e in range(N_CMD):
    _C1.append(_WP.alloc(3 + _e, f"c1_{_e}", KF * CTRL_H))
    _C2.append(_WP.alloc(3 + _e, f"c2_{_e}", 2 * CTRL_H))
    _C3.append(_WP.alloc(3 + _e, f"c3_{_e}", 2 * 4))
_WC = tuple(_WP.cur)


def _build_wpacks(i):
    ws = [np.zeros((P, c), dtype=np.float32) for c in _WC]

    def put_lhst(dst, mat, m_cols):
        pack, base = dst
        w = ws[pack]
        K = mat.shape[0]
        for k in range((K + P - 1) // P):
            blk = mat[k * P:(k + 1) * P]
            w[: blk.shape[0], base + k * m_cols: base + k * m_cols + mat.shape[1]] = blk

    def col(dst, extra=0):
        pack, base = dst
        return ws[pack], base + extra

    put_lhst(_MS1, np.asarray(i["ms_w1"], np.float32), P)
    put_lhst(_MS2, np.asarray(i["ms_w2"], np.float32), P)
    put_lhst(_MS3, np.asarray(i["ms_w3"], np.float32), P)
    put_lhst(_JOIN, np.asarray(i["join_w"], np.float32), FEAT)
    put_lhst(_SP1, np.asarray(i["sp_w1"], np.float32), CTRL_H)
    put_lhst(_SP2, np.asarray(i["sp_w2"], np.float32), CTRL_H)
    put_lhst(_SP3, np.asarray(i["sp_w3"], np.float32), 2)
    cw1 = np.asarray(i["ctrl_w1"], np.float32)
    cw2 = np.asarray(i["ctrl_w2"], np.float32)
    cw3 = np.asarray(i["ctrl_w3"], np.float32)
    for e in range(N_CMD):
        put_lhst(_C1[e], cw1[e], CTRL_H)
        put_lhst(_C2[e], cw2[e], CTRL_H)
        c3p = np.zeros((2 * P, 4), dtype=np.float32)
        c3p[:, 1:4] = cw3[e]
        put_lhst(_C3[e], c3p, 4)

    w, b = col(_B_MS1); w[:, b] = np.asarray(i["ms_b1"], np.float32)
    w, b = col(_B_MS2); w[:, b] = np.asarray(i["ms_b2"], np.float32)
    w, b = col(_B_MS3); w[:, b] = np.asarray(i["ms_b3"], np.float32)
    jb = np.asarray(i["join_b"], np.float32)
    sb1 = np.asarray(i["sp_b1"], np.float32)
    sb2 = np.asarray(i["sp_b2"], np.float32)
    for m in range(KF):
        w, b = col(_B_JOIN, m); w[:, b] = jb[m * P:(m + 1) * P]
    for m in range(2):
        w, b = col(_B_SP1, m); w[:, b] = sb1[m * P:(m + 1) * P]
        w, b = col(_B_SP2, m); w[:, b] = sb2[m * P:(m + 1) * P]
    w, b = col(_B_SP3); w[0, b] = np.asarray(i["sp_b3"], np.float32)[0]
    cb1 = np.asarray(i["ctrl_b1"], np.float32)
    cb2 = np.asarray(i["ctrl_b2"], np.float32)
    cb3 = np.asarray(i["ctrl_b3"], np.float32)
    for e in range(N_CMD):
        for m in range(2):
            w, b = col(_B_C1, e * 2 + m); w[:, b] = cb1[e, m * P:(m + 1) * P]
            w, b = col(_B_C2, e * 2 + m); w[:, b] = cb2[e, m * P:(m + 1) * P]
        w, b = col(_B_C3, e); w[1:4, b] = cb3[e]
    return [_round_fp32r(w) for w in ws]


# ---------------------------------------------------------------------------
# Bass kernel builder (cached per (T, head-boundary) signature).
# ---------------------------------------------------------------------------
@functools.lru_cache(maxsize=8)
def _build_nc(T: int, bounds: tuple):
    """bounds: (o1, o2, o3) column offsets where heads 1,2,3 start; head 0
    starts at 0, head 3 ends at T. All offsets are multiples of 256."""
    assert T % 2 == 0
    # chunk widths: full 512s, then split the remainder into 1-2 even chunks
    # of >=256 columns so every matmul keeps the fp32r full-rate window
    widths = []
    rem = T
    while rem >= CH + 256 or rem == CH:
        widths.append(CH)
        rem -= CH
    if rem > CH:
        a = (rem // 2 + 1) & ~1
        widths.append(a)
        widths.append(rem - a)
    elif rem > 0:
        widths.append(rem)
    starts = [0]
    for w in widths:
        starts.append(starts[-1] + w)
    n_chunks = len(widths)
    offs = (0,) + tuple(bounds) + (T,)
    # chunk whose front() should kick off head e's ctrl weight load
    gate_chunk = []
    for e in range(N_CMD):
        fc = 0
        while fc + 1 < n_chunks and starts[fc + 1] <= offs[e]:
            fc += 1
        gate_chunk.append(max(fc - 1, 0))

    nc = bacc.Bacc("TRN2", target_bir_lowering=False, debug=False,
                   num_devices=N_CORES)
    featd = nc.declare_dram_parameter("featT", [P, KF, T], F32R, isOutput=False)
    sprd = nc.declare_dram_parameter("sprow", [1, T], F32R, isOutput=False)
    wpd = [nc.declare_dram_parameter(f"wpack{k}", [P, _WC[k]], F32R,
                                     isOutput=False) for k in range(7)]
    outd = nc.declare_dram_parameter("out", [4, T], F32, isOutput=True)

    def segs_of(c):
        lo_c, hi_c = starts[c], starts[c + 1]
        out = []
        for e in range(N_CMD):
            lo, hi = max(lo_c, offs[e]), min(hi_c, offs[e + 1])
            if lo < hi:
                out.append((lo - lo_c, hi - lo_c, e))
        return out

    with tile.TileContext(nc) as tc, ExitStack() as ctx:
        wpool = ctx.enter_context(tc.tile_pool(name="w", bufs=1))
        fpool = ctx.enter_context(tc.tile_pool(name="f", bufs=3))
        apool = ctx.enter_context(tc.tile_pool(name="a", bufs=2))
        opool = ctx.enter_context(tc.tile_pool(name="o", bufs=3))
        pp = ctx.enter_context(tc.tile_pool(name="pp", bufs=8, space="PSUM"))

        spt = wpool.tile([1, T], F32R)
        nc.sync.dma_start(spt[:], sprd[:])
        wp = [wpool.tile([P, _WC[k]], F32R, name=f"wp{k}") for k in range(7)]
        # pack0 (small, needed by the first matmuls) loads up front; the big
        # packs are issued on the SWDGE ring inside chunk 0 so the first feat
        # chunks get the startup HBM bandwidth.
        nc.scalar.dma_start(wp[0][:], wpd[0][:])
        nc.scalar.dma_start(wp[1][:], wpd[1][:])
        nc.scalar.dma_start(wp[2][:], wpd[2][:])

        def wsl(dst, extra, m):
            pack, base = dst
            return wp[pack][:, base + extra: base + extra + m]

        def bias(dst, extra=0):
            pack, base = dst
            return wp[pack][:, base + extra: base + extra + 1].bitcast(F32)

        def brow(dst, extra, r0, r1):
            pack, base = dst
            return wp[pack][r0:r1, base + extra: base + extra + 1].bitcast(F32)

        npsum = [0]

        def psum(p_, n_):
            npsum[0] += 1
            return pp.tile([p_, n_], F32, tag="ps", name=f"ps{npsum[0]}",
                           padded_shape=[P, CH])

        mm = nc.tensor.matmul
        act = nc.scalar.activation

        st = {}  # per-chunk carried state


        def front(c):
            """ms/sp/join matmuls of chunk c, with hooks for ctrl(c-1)."""
            cw = widths[c]
            sl = slice(starts[c], starts[c + 1])
            fa = fpool.tile([P, KF, CH], F32R, tag="feat", name=f"fa{c}")
            nc.sync.dma_start(fa[:, 0:2, :cw], featd[:, 0:2, sl])
            nc.sync.dma_start(fa[:, 2:4, :cw], featd[:, 2:4, sl])
            s = st[c] = {}

            # A: ms1 (outer product from speed row)
            p = psum(P, cw)
            mm(p[:], wp[0][0:1, _MS1[1]:_MS1[1] + P], spt[0:1, sl],
               start=True, stop=True)
            h1 = apool.tile([P, CH], F32R, tag="h1", name=f"h1_{c}")
            act(h1[:, :cw], p[:], AF.Relu, bias=bias(_B_MS1))

            # B: sp1
            s1 = []
            for m in range(2):
                p = psum(P, cw)
                for k in range(KF):
                    mm(p[:], wsl(_SP1, k * CTRL_H + m * P, P), fa[:, k, :cw],
                       start=(k == 0), stop=(k == KF - 1))
                t = apool.tile([P, CH], F32R, tag=f"s1{m}", name=f"s1{m}_{c}")
                act(t[:, :cw], p[:], AF.Relu, bias=bias(_B_SP1, m))
                s1.append(t)

            # D: ms2
            p = psum(P, cw)
            mm(p[:], wsl(_MS2, 0, P), h1[:, :cw], start=True, stop=True)
            h2 = apool.tile([P, CH], F32R, tag="h2", name=f"h2_{c}")
            act(h2[:, :cw], p[:], AF.Relu, bias=bias(_B_MS2))

            # C: ctrl1 of previous chunk
            ctrl1(c - 1)

            # G: ms3
            p = psum(P, cw)
            mm(p[:], wsl(_MS3, 0, P), h2[:, :cw], start=True, stop=True)
            v = apool.tile([P, CH], F32R, tag="v", name=f"v_{c}")
            act(v[:, :cw], p[:], AF.Identity, bias=bias(_B_MS3))

            # H: sp2
            s2 = []
            for m in range(2):
                p = psum(P, cw)
                for k in range(2):
                    mm(p[:], wsl(_SP2, k * CTRL_H + m * P, P), s1[k][:, :cw],
                       start=(k == 0), stop=(k == 1))
                t = apool.tile([P, CH], F32R, tag=f"s2{m}", name=f"s2{m}_{c}")
                act(t[:, :cw], p[:], AF.Relu, bias=bias(_B_SP2, m))
                s2.append(t)
            # stagger the per-head ctrl weight loads: each is gated (WAW
            # touch) behind this chunk's data so startup DMAs keep priority
            for e in range(N_CMD):
                if gate_chunk[e] == c:
                    if c == 0:
                        nc.vector.tensor_copy(wp[3 + e][0:1, 0:2],
                                              wp[2][0:1, 0:2])
                    else:
                        nc.vector.tensor_copy(wp[3 + e][0:1, 0:2],
                                              fa[0:1, 0, 0:2])
                    nc.gpsimd.dma_start(wp[3 + e][:], wpd[3 + e][:])

            # F: ctrl2 of previous chunk
            ctrl2(c - 1)

            # E+I: join (k0..3 from feat, then k4 from v)
            jps = []
            for m in range(KF):
                p = psum(P, cw)
                for k in range(KF):
                    mm(p[:], wsl(_JOIN, k * FEAT + m * P, P), fa[:, k, :cw],
                       start=(k == 0), stop=False)
                jps.append(p)
            j = []
            for m in range(KF):
                mm(jps[m][:], wsl(_JOIN, KF * FEAT + m * P, P), v[:, :cw],
                   start=False, stop=True)
                t = apool.tile([P, CH], F32R, tag=f"j{m}", name=f"j{m}_{c}")
                nc.vector.tensor_scalar_add(t[:, :cw], jps[m][:], bias(_B_JOIN, m))
                j.append(t)
            s["j"] = j

            # J: ctrl3 of previous chunk (+ its output DMAs)
            ctrl3(c - 1)

            # K: sp3
            p3 = psum(2, cw)
            for k in range(2):
                mm(p3[:], wsl(_SP3, 2 * k, 2), s2[k][:, :cw],
                   start=(k == 0), stop=(k == 1))
            vp_t = apool.tile([1, CH], F32, tag="vp", name=f"vp_{c}")
            act(vp_t[0:1, :cw], p3[0:1, :], AF.Identity, bias=brow(_B_SP3, 0, 0, 1))
            s["vp"] = vp_t

        def ctrl1(c):
            if c < 0:
                return
            s = st[c]
            j = s["j"]
            s["t1"] = {}
            for (lo, hi, e) in segs_of(c):
                n = hi - lo
                t1 = []
                for m in range(2):
                    p = psum(P, n)
                    for k in range(KF):
                        mm(p[:], wsl(_C1[e], k * CTRL_H + m * P, P),
                           j[k][:, lo:hi],
                           start=(k == 0), stop=(k == KF - 1))
                    t = apool.tile([P, CH], F32R, tag=f"t1{m}",
                                   name=f"t1{m}_{c}_{lo}")
                    nc.vector.tensor_scalar(t[:, :n], p[:],
                                            bias(_B_C1, e * 2 + m), 0.0,
                                            ALU.add, ALU.max)
                    t1.append(t)
                s["t1"][lo] = t1

        def ctrl2(c):
            if c < 0:
                return
            s = st[c]
            s["t2"] = {}
            for (lo, hi, e) in segs_of(c):
                n = hi - lo
                t1 = s["t1"][lo]
                t2 = []
                for m in range(2):
                    p = psum(P, n)
                    for k in range(2):
                        mm(p[:], wsl(_C2[e], k * CTRL_H + m * P, P),
                           t1[k][:, :n], start=(k == 0), stop=(k == 1))
                    t = apool.tile([P, CH], F32R, tag=f"t2{m}",
                                   name=f"t2{m}_{c}_{lo}")
                    nc.vector.tensor_scalar(t[:, :n], p[:],
                                            bias(_B_C2, e * 2 + m), 0.0,
                                            ALU.add, ALU.max)
                    t2.append(t)
                s["t2"][lo] = t2

        def ctrl3(c):
            if c < 0:
                return
            s = st[c]
            cw = widths[c]
            sl = slice(starts[c], starts[c + 1])
            outS = opool.tile([4, CH], F32, tag="outS", name=f"outS_{c}")
            outT = opool.tile([4, CH], F32, tag="outT", name=f"outT_{c}")
            for (lo, hi, e) in segs_of(c):
                n = hi - lo
                t2 = s["t2"][lo]
                pc = psum(4, n)
                for k in range(2):
                    mm(pc[:], wsl(_C3[e], k * 4, 4), t2[k][:, :n],
                       start=(k == 0), stop=(k == 1))
                # psum rows: 0=pad, 1=act0(throttle), 2=act1(steer), 3=act2(brake)
                act(outS[0:4, lo:hi], pc[0:4, :], AF.Sigmoid,
                    bias=brow(_B_C3, e, 0, 4))
                act(outT[0:4, lo:hi], pc[0:4, :], AF.Tanh,
                    bias=brow(_B_C3, e, 0, 4))
            # out rows: 0=v_p, 1=throttle, 2=steering, 3=brake
            nc.sync.dma_start(outd[0:1, sl], st[c]["vp"][0:1, :cw])
            nc.sync.dma_start(outd[1:2, sl], outS[1:2, :cw])
            nc.sync.dma_start(outd[2:3, sl], outT[2:3, :cw])
            nc.sync.dma_start(outd[3:4, sl], outS[3:4, :cw])
            del st[c]["t1"], st[c]["t2"], st[c]["j"]

        for c in range(n_chunks):
            front(c)
        ctrl1(n_chunks - 1)
        ctrl2(n_chunks - 1)
        ctrl3(n_chunks - 1)

    nc.compile()
    return nc


def _roundup(x, m):
    return (x + m - 1) // m * m


def kernel(**inputs) -> tuple:
    global LAST_RESULTS
    feat = np.asarray(inputs["feat"], np.float32)
    speed = np.asarray(inputs["speed"], np.float32)
    command = np.asarray(inputs["command"]).astype(np.int64)
    B = feat.shape[0]

    # ---- host routing: group rows by command, deal round-robin over cores --
    per_core_groups = [[None] * N_CMD for _ in range(N_CORES)]
    for e in range(N_CMD):
        idx = np.nonzero(command == e)[0]
        for cid in range(N_CORES):
            per_core_groups[cid][e] = idx[cid::N_CORES]

    counts = np.array([[len(per_core_groups[cid][e]) for e in range(N_CMD)]
                       for cid in range(N_CORES)])
    caps = counts.max(axis=0)  # per-head capacity across cores
    # Head boundaries: even columns; avoid splitting a chunk into two
    # mid-sized pieces (both would lose the fp32r full-rate N>=256 window) by
    # pushing such boundaries up to the next 256 multiple.
    offs = [0]
    for e in range(N_CMD):
        b = _roundup(offs[e] + int(caps[e]), 2)
        if 130 < (b % CH) < 382:
            b = _roundup(b, 256)
        offs.append(b)
    T = max(_roundup(offs[N_CMD], 2), 512)
    bounds = tuple(offs[1:4])

    wpacks = _build_wpacks(inputs)

    in_maps = []
    for cid in range(N_CORES):
        rows = np.zeros((T, FEAT), dtype=np.float32)
        spr = np.zeros(T, dtype=np.float32)
        for e in range(N_CMD):
            g = per_core_groups[cid][e]
            rows[offs[e]: offs[e] + len(g)] = feat[g]
            spr[offs[e]: offs[e] + len(g)] = speed[g]
        featT = _round_fp32r(rows.T)                      # [512, T]
        featd = np.ascontiguousarray(
            featT.reshape(KF, P, T).transpose(1, 0, 2))   # [128, 4, T]
        in_maps.append({
            "featT": featd,
            "sprow": _round_fp32r(spr)[None, :],
            **{f"wpack{k}": wpacks[k] for k in range(7)},
        })

    nc = _build_nc(T, bounds)
    trace = os.environ.get("KERNEL_TRACE", "") == "1"
    res = run_bass_kernel_spmd(nc, in_maps, core_ids=list(range(N_CORES)),
                               trace=trace)
    LAST_RESULTS = res

    v_p = np.zeros((B, 1), dtype=np.float32)
    throttle = np.zeros(B, dtype=np.float32)
    brake = np.zeros(B, dtype=np.float32)
    steering = np.zeros(B, dtype=np.float32)
    for cid in range(N_CORES):
        o = res.results[cid]["out"]  # [4, T]
        for e in range(N_CMD):
            g = per_core_groups[cid][e]
            sl = slice(offs[e], offs[e] + len(g))
            v_p[g, 0] = o[0, sl]
            throttle[g] = o[1, sl]
            steering[g] = o[2, sl]
            brake[g] = o[3, sl]
    return v_p, throttle, brake, steering


# revision 20
# speedup vs baseline: 1.5082x; 1.4274x over previous
"""Trainium2 Bass kernel for the CILRS-style command-conditioned driving head.

Strategy (pure data parallel across 8 NeuronCores + host-side MoE routing):
  - Rows are grouped by `command` on the host and dealt round-robin across the
    8 cores, so each core gets ~B/8 rows with 4 contiguous command groups
    (boundaries padded to even columns; a boundary that would split a chunk
    into two mid-sized pieces is pushed to a 256 multiple so fp32r matmuls
    keep their full-rate N>=256 window).
  - Everything on device is computed feature-major ([feature, batch]): the
    batch lives on the PE moving (free) axis, weights are the stationary
    operand in their natural [in, out] layout, so no transposes are needed
    anywhere on device (feat is transposed once on the host).
  - Linear-layer fusion: `joined = [feat, v] @ join_w + join_b` and
    `v = h2 @ ms_w3 + ms_b3` are both linear, and `joined` is consumed only
    by the ctrl heads. The host folds join+ms3 into each head's first layer:
        h1_e = relu(feat @ (Jf @ C1_e) + h2 @ (ms_w3 @ Jv @ C1_e) + b_e)
    eliminating the join stage and ms3 entirely (per-512-column PE passes
    drop from 51 to 32).
  - All matmuls run in float32r (TF32-like, ~1e-4 relative rounding) at the
    full 1 element/cycle PE rate. PSUM accumulates in fp32. Bias + ReLU
    epilogues are split across the Activation and Vector engines and write
    float32r so downstream matmuls can legally consume them.
  - Software pipelining: the ctrl-head stages of chunk c-1 are interleaved
    into chunk c's ms/sp matmul stream so the PE never stalls on an epilogue
    and the HAM clock gate stays warm. Weight DMAs are split into 7 packs:
    the startup-critical ones load up front on the scalar HWDGE ring; the
    four per-head ctrl packs are gated (WAW touch) to load just-in-time.

kernel(**inputs) takes the FULL unsharded inputs (as produced by
reference.setup_inputs()) and returns (v_p [B,1], throttle [B], brake [B],
steering [B]) as float32, matching the reference output tuple.
"""

import functools
import os
from contextlib import ExitStack

import numpy as np

import concourse.bacc as bacc
import concourse.tile as tile
from concourse import mybir
from concourse.bass_utils import run_bass_kernel_spmd

N_CORES = 8
P = 128
CH = 512  # batch-column chunk width (one PSUM bank of fp32)
FEAT, SPD_H, CTRL_H, N_CMD = 512, 128, 256, 4
KF = FEAT // P  # feat k-tiles (4)
F32 = mybir.dt.float32
F32R = mybir.dt.float32r

AF = mybir.ActivationFunctionType
ALU = mybir.AluOpType

# Results of the most recent kernel() call (for test harness introspection).
LAST_RESULTS = None


def _round_fp32r(x: np.ndarray) -> np.ndarray:
    """Round-to-nearest fp32 -> fp32r (11-bit mantissa, low 12 bits zero)."""
    u = np.ascontiguousarray(x, dtype=np.float32).view(np.uint32).astype(np.uint64)
    r = (u + 0x800) & 0xFFFF_F000
    return r.astype(np.uint32).view(np.float32)


# ---------------------------------------------------------------------------
# Weight packs, addressed as (pack, col):
#   pack 0: ms1, ms2 + all biases (tiny, needed first)
#   pack 1: sp1
#   pack 2: sp2, sp3
#   packs 3..6: ctrl head e = {c1 feat-part, c1 h2-part, c2, c3}
# ---------------------------------------------------------------------------
class _WPack:
    def __init__(self):
        self.cur = [0] * 7
        self.off = {}

    def alloc(self, pack, name, cols):
        self.off[name] = (pack, self.cur[pack])
        self.cur[pack] += cols
        return self.off[name]


_WP = _WPack()
_MS1 = _WP.alloc(0, "ms1", P)           # [1,128] in row 0
_MS2 = _WP.alloc(0, "ms2", P)
_B_MS1 = _WP.alloc(0, "b_ms1", 1)
_B_MS2 = _WP.alloc(0, "b_ms2", 1)
_B_SP1 = _WP.alloc(0, "b_sp1", 2)
_B_SP2 = _WP.alloc(0, "b_sp2", 2)
_B_SP3 = _WP.alloc(0, "b_sp3", 1)
_B_C1 = _WP.alloc(0, "b_c1", N_CMD * 2)  # fused ctrl1 bias
_B_C2 = _WP.alloc(0, "b_c2", N_CMD * 2)
_B_C3 = _WP.alloc(0, "b_c3", N_CMD)      # rows 1..3 (row 0 = pad lane)
_SP1 = _WP.alloc(1, "sp1", KF * CTRL_H)
_SP2 = _WP.alloc(2, "sp2", 2 * CTRL_H)
_SP3 = _WP.alloc(2, "sp3", 2 * 2)
_C1F, _C1H, _C2, _C3 = [], [], [], []
for _e in range(N_CMD):
    _C1F.append(_WP.alloc(3 + _e, f"c1f_{_e}", KF * CTRL_H))  # feat part
    _C1H.append(_WP.alloc(3 + _e, f"c1h_{_e}", CTRL_H))       # h2 part
    _C2.append(_WP.alloc(3 + _e, f"c2_{_e}", 2 * CTRL_H))
    _C3.append(_WP.alloc(3 + _e, f"c3_{_e}", 2 * 4))
_WC = tuple(_WP.cur)


def _build_wpacks(i):
    ws = [np.zeros((P, c), dtype=np.float32) for c in _WC]

    def put_lhst(dst, mat, m_cols):
        pack, base = dst
        w = ws[pack]
        K = mat.shape[0]
        for k in range((K + P - 1) // P):
            blk = mat[k * P:(k + 1) * P]
            w[: blk.shape[0], base + k * m_cols: base + k * m_cols + mat.shape[1]] = blk

    def col(dst, extra=0):
        pack, base = dst
        return ws[pack], base + extra

    f64 = lambda x: np.asarray(i[x], np.float64)
    put_lhst(_MS1, np.asarray(i["ms_w1"], np.float32), P)
    put_lhst(_MS2, np.asarray(i["ms_w2"], np.float32), P)
    put_lhst(_SP1, np.asarray(i["sp_w1"], np.float32), CTRL_H)
    put_lhst(_SP2, np.asarray(i["sp_w2"], np.float32), CTRL_H)
    put_lhst(_SP3, np.asarray(i["sp_w3"], np.float32), 2)

    # --- fold join + ms3 into each ctrl head's first layer (all linear) ---
    Jf = f64("join_w")[:FEAT]        # [512, 512]
    Jv = f64("join_w")[FEAT:]        # [128, 512]
    ms_w3, ms_b3 = f64("ms_w3"), f64("ms_b3")
    join_b = f64("join_b")
    w_h2 = ms_w3 @ Jv                # [128, 512]
    b_lin = ms_b3 @ Jv + join_b      # [512]
    cw1, cb1 = f64("ctrl_w1"), f64("ctrl_b1")
    cw2 = np.asarray(i["ctrl_w2"], np.float32)
    cw3 = np.asarray(i["ctrl_w3"], np.float32)
    cb3 = np.asarray(i["ctrl_b3"], np.float32)
    for e in range(N_CMD):
        put_lhst(_C1F[e], (Jf @ cw1[e]).astype(np.float32), CTRL_H)
        put_lhst(_C1H[e], (w_h2 @ cw1[e]).astype(np.float32), CTRL_H)
        put_lhst(_C2[e], cw2[e], CTRL_H)
        c3p = np.zeros((2 * P, 4), dtype=np.float32)
        c3p[:, 1:4] = cw3[e]
        put_lhst(_C3[e], c3p, 4)
        bc1 = (b_lin @ cw1[e] + cb1[e]).astype(np.float32)
        for m in range(2):
            w, b = col(_B_C1, e * 2 + m); w[:, b] = bc1[m * P:(m + 1) * P]

    w, b = col(_B_MS1); w[:, b] = np.asarray(i["ms_b1"], np.float32)
    w, b = col(_B_MS2); w[:, b] = np.asarray(i["ms_b2"], np.float32)
    sb1 = np.asarray(i["sp_b1"], np.float32)
    sb2 = np.asarray(i["sp_b2"], np.float32)
    for m in range(2):
        w, b = col(_B_SP1, m); w[:, b] = sb1[m * P:(m + 1) * P]
        w, b = col(_B_SP2, m); w[:, b] = sb2[m * P:(m + 1) * P]
    w, b = col(_B_SP3); w[0, b] = np.asarray(i["sp_b3"], np.float32)[0]
    cb2 = np.asarray(i["ctrl_b2"], np.float32)
    for e in range(N_CMD):
        for m in range(2):
            w, b = col(_B_C2, e * 2 + m); w[:, b] = cb2[e, m * P:(m + 1) * P]
        w, b = col(_B_C3, e); w[1:4, b] = cb3[e]
    return [_round_fp32r(w) for w in ws]


# ---------------------------------------------------------------------------
# Bass kernel builder (cached per (T, head-boundary) signature).
# ---------------------------------------------------------------------------
@functools.lru_cache(maxsize=8)
def _build_nc(T: int, bounds: tuple):
    """bounds: (o1, o2, o3) even column offsets where heads 1,2,3 start;
    head 0 starts at 0, head 3 ends at T."""
    assert T % 2 == 0
    # chunk widths: full 512s, then split the remainder into 1-2 even chunks
    # of >=256 columns so every matmul keeps the fp32r full-rate window
    widths = []
    rem = T
    while rem >= CH + 256 or rem == CH:
        widths.append(CH)
        rem -= CH
    if rem > CH:
        a = (rem // 2 + 1) & ~1
        widths.append(a)
        widths.append(rem - a)
    elif rem > 0:
        widths.append(rem)
    starts = [0]
    for w in widths:
        starts.append(starts[-1] + w)
    n_chunks = len(widths)
    offs = (0,) + tuple(bounds) + (T,)
    # chunk whose front() should kick off head e's ctrl weight load
    gate_chunk = []
    for e in range(N_CMD):
        fc = 0
        while fc + 1 < n_chunks and starts[fc + 1] <= offs[e]:
            fc += 1
        gate_chunk.append(max(fc - 1, 0))

    nc = bacc.Bacc("TRN2", target_bir_lowering=False, debug=False,
                   num_devices=N_CORES)
    featd = nc.declare_dram_parameter("featT", [P, KF, T], F32R, isOutput=False)
    sprd = nc.declare_dram_parameter("sprow", [1, T], F32R, isOutput=False)
    wpd = [nc.declare_dram_parameter(f"wpack{k}", [P, _WC[k]], F32R,
                                     isOutput=False) for k in range(7)]
    outd = nc.declare_dram_parameter("out", [4, T], F32, isOutput=True)

    def segs_of(c):
        """(lo, hi, head) in chunk-relative columns."""
        lo_c, hi_c = starts[c], starts[c + 1]
        out = []
        for e in range(N_CMD):
            lo, hi = max(lo_c, offs[e]), min(hi_c, offs[e + 1])
            if lo < hi:
                out.append((lo - lo_c, hi - lo_c, e))
        return out

    with tile.TileContext(nc) as tc, ExitStack() as ctx:
        wpool = ctx.enter_context(tc.tile_pool(name="w", bufs=1))
        fpool = ctx.enter_context(tc.tile_pool(name="f", bufs=3))
        apool = ctx.enter_context(tc.tile_pool(name="a", bufs=2))
        opool = ctx.enter_context(tc.tile_pool(name="o", bufs=3))
        pp = ctx.enter_context(tc.tile_pool(name="pp", bufs=8, space="PSUM"))

        spt = wpool.tile([1, T], F32R)
        nc.sync.dma_start(spt[:], sprd[:])
        wp = [wpool.tile([P, _WC[k]], F32R, name=f"wp{k}") for k in range(7)]
        # startup-critical packs load up front on the scalar HWDGE ring (the
        # sync ring carries the feat chunks); per-head ctrl packs are gated.
        for k in range(3):
            nc.scalar.dma_start(wp[k][:], wpd[k][:])

        def wsl(dst, extra, m):
            pack, base = dst
            return wp[pack][:, base + extra: base + extra + m]

        def bias(dst, extra=0):
            pack, base = dst
            return wp[pack][:, base + extra: base + extra + 1].bitcast(F32)

        def brow(dst, extra, r0, r1):
            pack, base = dst
            return wp[pack][r0:r1, base + extra: base + extra + 1].bitcast(F32)

        npsum = [0]

        def psum(p_, n_):
            npsum[0] += 1
            return pp.tile([p_, n_], F32, tag="ps", name=f"ps{npsum[0]}",
                           padded_shape=[P, CH])

        mm = nc.tensor.matmul
        act = nc.scalar.activation

        st = {}  # per-chunk carried state

        def front(c):
            cw = widths[c]
            sl = slice(starts[c], starts[c + 1])
            fa = fpool.tile([P, KF, CH], F32R, tag="feat", name=f"fa{c}")
            nc.sync.dma_start(fa[:, 0:2, :cw], featd[:, 0:2, sl])
            nc.sync.dma_start(fa[:, 2:4, :cw], featd[:, 2:4, sl])
            s = st[c] = {"fa": fa, "cw": cw}

            # A: ms1 (outer product from speed row)
            p = psum(P, cw)
            mm(p[:], wp[0][0:1, _MS1[1]:_MS1[1] + P], spt[0:1, sl],
               start=True, stop=True)
            h1 = apool.tile([P, CH], F32R, tag="h1", name=f"h1_{c}")
            act(h1[:, :cw], p[:], AF.Relu, bias=bias(_B_MS1))

            # B: sp1
            s1 = []
            for m in range(2):
                p = psum(P, cw)
                for k in range(KF):
                    mm(p[:], wsl(_SP1, k * CTRL_H + m * P, P), fa[:, k, :cw],
                       start=(k == 0), stop=(k == KF - 1))
                t = apool.tile([P, CH], F32R, tag=f"s1{m}", name=f"s1{m}_{c}")
                act(t[:, :cw], p[:], AF.Relu, bias=bias(_B_SP1, m))
                s1.append(t)

            # stagger the per-head ctrl weight loads: gated (WAW touch)
            # behind this chunk's feat so startup DMAs keep priority
            for e in range(N_CMD):
                if gate_chunk[e] == c:
                    if c == 0:
                        nc.vector.tensor_copy(wp[3 + e][0:1, 0:2],
                                              wp[2][0:1, 0:2])
                    else:
                        nc.vector.tensor_copy(wp[3 + e][0:1, 0:2],
                                              fa[0:1, 0, 0:2])
                    nc.gpsimd.dma_start(wp[3 + e][:], wpd[3 + e][:])

            # D: ms2 -> h2 (consumed by the fused ctrl1 of this chunk)
            p = psum(P, cw)
            mm(p[:], wsl(_MS2, 0, P), h1[:, :cw], start=True, stop=True)
            h2 = apool.tile([P, CH], F32R, tag="h2", name=f"h2_{c}")
            act(h2[:, :cw], p[:], AF.Relu, bias=bias(_B_MS2))
            s["h2"] = h2

            # C: ctrl1 of previous chunk (fused join+ms3+ctrl1)
            ctrl1(c - 1)

            # H: sp2
            s2 = []
            for m in range(2):
                p = psum(P, cw)
                for k in range(2):
                    mm(p[:], wsl(_SP2, k * CTRL_H + m * P, P), s1[k][:, :cw],
                       start=(k == 0), stop=(k == 1))
                t = apool.tile([P, CH], F32R, tag=f"s2{m}", name=f"s2{m}_{c}")
                act(t[:, :cw], p[:], AF.Relu, bias=bias(_B_SP2, m))
                s2.append(t)

            # F: ctrl2 of previous chunk
            ctrl2(c - 1)

            # K: sp3
            p3 = psum(2, cw)
            for k in range(2):
                mm(p3[:], wsl(_SP3, 2 * k, 2), s2[k][:, :cw],
                   start=(k == 0), stop=(k == 1))
            vp_t = apool.tile([1, CH], F32, tag="vp", name=f"vp_{c}")
            act(vp_t[0:1, :cw], p3[0:1, :], AF.Identity, bias=brow(_B_SP3, 0, 0, 1))
            s["vp"] = vp_t

            # J: ctrl3 of previous chunk (+ its output DMAs)
            ctrl3(c - 1)

        def ctrl1(c):
            if c < 0:
                return
            s = st[c]
            fa, h2 = s["fa"], s["h2"]
            s["t1"] = {}
            for (lo, hi, e) in segs_of(c):
                n = hi - lo
                t1 = []
                for m in range(2):
                    p = psum(P, n)
                    for k in range(KF):
                        mm(p[:], wsl(_C1F[e], k * CTRL_H + m * P, P),
                           fa[:, k, lo:hi], start=(k == 0), stop=False)
                    mm(p[:], wsl(_C1H[e], m * P, P), h2[:, lo:hi],
                       start=False, stop=True)
                    t = apool.tile([P, CH], F32R, tag=f"t1{m}",
                                   name=f"t1{m}_{c}_{lo}")
                    nc.vector.tensor_scalar(t[:, :n], p[:],
                                            bias(_B_C1, e * 2 + m), 0.0,
                                            ALU.add, ALU.max)
                    t1.append(t)
                s["t1"][lo] = t1

        def ctrl2(c):
            if c < 0:
                return
            s = st[c]
            s["t2"] = {}
            for (lo, hi, e) in segs_of(c):
                n = hi - lo
                t1 = s["t1"][lo]
                t2 = []
                for m in range(2):
                    p = psum(P, n)
                    for k in range(2):
                        mm(p[:], wsl(_C2[e], k * CTRL_H + m * P, P),
                           t1[k][:, :n], start=(k == 0), stop=(k == 1))
                    t = apool.tile([P, CH], F32R, tag=f"t2{m}",
                                   name=f"t2{m}_{c}_{lo}")
                    nc.vector.tensor_scalar(t[:, :n], p[:],
                                            bias(_B_C2, e * 2 + m), 0.0,
                                            ALU.add, ALU.max)
                    t2.append(t)
                s["t2"][lo] = t2

        def ctrl3(c):
            if c < 0:
                return
            s = st[c]
            cw = s["cw"]
            sl = slice(starts[c], starts[c + 1])
            outS = opool.tile([4, CH], F32, tag="outS", name=f"outS_{c}")
            outT = opool.tile([4, CH], F32, tag="outT", name=f"outT_{c}")
            for (lo, hi, e) in segs_of(c):
                n = hi - lo
                t2 = s["t2"][lo]
                pc = psum(4, n)
                for k in range(2):
                    mm(pc[:], wsl(_C3[e], k * 4, 4), t2[k][:, :n],
                       start=(k == 0), stop=(k == 1))
                # psum rows: 0=pad, 1=act0(throttle), 2=act1(steer), 3=act2(brake)
                act(outS[0:4, lo:hi], pc[0:4, :], AF.Sigmoid,
                    bias=brow(_B_C3, e, 0, 4))
                act(outT[0:4, lo:hi], pc[0:4, :], AF.Tanh,
                    bias=brow(_B_C3, e, 0, 4))
            # out rows: 0=v_p, 1=throttle, 2=steering, 3=brake
            nc.sync.dma_start(outd[0:1, sl], s["vp"][0:1, :cw])
            nc.sync.dma_start(outd[1:2, sl], outS[1:2, :cw])
            nc.sync.dma_start(outd[2:3, sl], outT[2:3, :cw])
            nc.sync.dma_start(outd[3:4, sl], outS[3:4, :cw])
            del s["t1"], s["t2"], s["fa"], s["h2"]

        for c in range(n_chunks):
            front(c)
        ctrl1(n_chunks - 1)
        ctrl2(n_chunks - 1)
        ctrl3(n_chunks - 1)

    nc.compile()
    return nc


def _roundup(x, m):
    return (x + m - 1) // m * m


def kernel(**inputs) -> tuple:
    global LAST_RESULTS
    feat = np.asarray(inputs["feat"], np.float32)
    speed = np.asarray(inputs["speed"], np.float32)
    command = np.asarray(inputs["command"]).astype(np.int64)
    B = feat.shape[0]

    # ---- host routing: group rows by command, deal round-robin over cores --
    per_core_groups = [[None] * N_CMD for _ in range(N_CORES)]
    for e in range(N_CMD):
        idx = np.nonzero(command == e)[0]
        for cid in range(N_CORES):
            per_core_groups[cid][e] = idx[cid::N_CORES]

    counts = np.array([[len(per_core_groups[cid][e]) for e in range(N_CMD)]
                       for cid in range(N_CORES)])
    caps = counts.max(axis=0)  # per-head capacity across cores
    # Head boundaries: even columns; avoid splitting a chunk into two
    # mid-sized pieces (both would lose the fp32r full-rate N>=256 window) by
    # pushing such boundaries up to the next 256 multiple.
    offs = [0]
    for e in range(N_CMD):
        b = _roundup(offs[e] + int(caps[e]), 2)
        if 130 < (b % CH) < 382:
            b = _roundup(b, 256)
        offs.append(b)
    T = max(_roundup(offs[N_CMD], 2), 512)
    bounds = tuple(offs[1:4])

    wpacks = _build_wpacks(inputs)

    in_maps = []
    for cid in range(N_CORES):
        rows = np.zeros((T, FEAT), dtype=np.float32)
        spr = np.zeros(T, dtype=np.float32)
        for e in range(N_CMD):
            g = per_core_groups[cid][e]
            rows[offs[e]: offs[e] + len(g)] = feat[g]
            spr[offs[e]: offs[e] + len(g)] = speed[g]
        featT = _round_fp32r(rows.T)                      # [512, T]
        featd = np.ascontiguousarray(
            featT.reshape(KF, P, T).transpose(1, 0, 2))   # [128, 4, T]
        in_maps.append({
            "featT": featd,
            "sprow": _round_fp32r(spr)[None, :],
            **{f"wpack{k}": wpacks[k] for k in range(7)},
        })

    nc = _build_nc(T, bounds)
    trace = os.environ.get("KERNEL_TRACE", "") == "1"
    res = run_bass_kernel_spmd(nc, in_maps, core_ids=list(range(N_CORES)),
                               trace=trace)
    LAST_RESULTS = res

    v_p = np.zeros((B, 1), dtype=np.float32)
    throttle = np.zeros(B, dtype=np.float32)
    brake = np.zeros(B, dtype=np.float32)
    steering = np.zeros(B, dtype=np.float32)
    for cid in range(N_CORES):
        o = res.results[cid]["out"]  # [4, T]
        for e in range(N_CMD):
            g = per_core_groups[cid][e]
            sl = slice(offs[e], offs[e] + len(g))
            v_p[g, 0] = o[0, sl]
            throttle[g] = o[1, sl]
            steering[g] = o[2, sl]
            brake[g] = o[3, sl]
    return v_p, throttle, brake, steering
